# revision 10
# baseline (speedup 1.0000x reference)
"""Trainium2 Bass kernel for a 3-layer BiLSTM + ReLU + residual + LayerNorm.

Pure data parallel over 8 cores (1024 batch rows per core).  Per core:

  * Transposed on-chip layout: features on partitions (fwd 0:64 / bwd 64:128
    fused), batch on the free dim, processed as two interleaved 512-row
    chunks so the engines ping-pong across the sequential dependency chain.
  * All matmuls are bf16 (1 cycle/row on the PE): the per-gate recurrent
    matmul is one block-diagonal 128x128; the layer-0 input projection is a
    single K=18 matmul per gate (both directions + bias rows folded in);
    layers 1-2 use two M=64 column-half matmuls per gate.  Recurrent
    matmuls are emitted at the head of each step and the next step's
    projections are emitted after the activations, so the PE queue is never
    head-of-line blocked on the h(t-1) dependency.
  * ScalarE runs the 4 gate activations (per-gate bias APs, sigmoid/tanh)
    plus tanh(c); i*g runs on GpSimd; f*c, the accumulate, and h=o*tanh(c)
    run on VectorE.  h is produced directly as bf16 into an 8-step staging
    ring that doubles as the recurrent-matmul rhs and the layer-output DMA
    source.
  * All HBM traffic is strip-batched (8 timesteps per DMA, backward halves
    via negative-stride APs): ~300 DMAs total instead of ~2300, which keeps
    the SP sequencer and the shared HWDGE path off the critical path.
  * Final stage per 8-step strip: residual via one K=9 matmul, z = relu+res
    on VectorE (bf16), z^2 on GpSimd, LN mean/sq-mean via ones-column
    accumulating matmuls (pre-scaled 1/128), tiny PE transposes to move the
    stats to natural layout, then a per-batch-row normalize split between
    ScalarE (scale/bias APs) and VectorE reading the PE-transposed z
    directly from PSUM.  relu runs one strip ahead on GpSimd; the two
    chunks' z-chains are interleaved per step (independent PSUM tiles) and
    the LN-sum matmuls lag two steps so they never head-of-line block the
    next residual matmul on the PE queue.

Cost-model time: ~1.64 ms vs the 3.27 ms fp32 baseline; max rel err ~7e-3
(bf16 recurrence) against the fp32 reference, within the 2e-2 gate.
"""

from contextlib import ExitStack

import numpy as np
import ml_dtypes

import concourse.bacc as bacc
import concourse.tile as tile
from concourse import mybir
from concourse.bass_utils import run_bass_kernel_spmd

F32 = mybir.dt.float32
BF16 = mybir.dt.bfloat16
AF = mybir.ActivationFunctionType
OP = mybir.AluOpType

NCORES = 8
BC = 1024               # batch rows per core
CHUNKS = 2
T = 64
H = 64
NL = 3
D2 = 2 * H              # 128
LN_EPS = 1e-5
SG = 8                  # timesteps per DMA strip group

# PyTorch gate order: i, f, g, o
GI, GF, GG, GO = 0, 1, 2, 3


def _host_prep(x, w_ih, w_hh, b_ih, b_hh, w_res, b_res, ncores, bc):
    """Matmul-ready weight layouts (shared across cores) + per-core inputs."""
    x = np.asarray(x, np.float32)
    w_ih = np.asarray(w_ih, np.float32)
    w_hh = np.asarray(w_hh, np.float32)
    bias = np.asarray(b_ih, np.float32) + np.asarray(b_hh, np.float32)  # (NL,2,4H)
    w_res = np.asarray(w_res, np.float32)
    b_res = np.asarray(b_res, np.float32)
    t_len = x.shape[1]

    # Recurrent lhsT, K-major: rw[k, l, g, m] (block-diagonal over directions)
    rw = np.zeros((128, NL, 4, 128), np.float32)
    for l in range(NL):
        for g in range(4):
            gs = slice(g * H, (g + 1) * H)
            rw[0:64, l, g, 0:64] = w_hh[l, 0, gs, :].T
            rw[64:128, l, g, 64:128] = w_hh[l, 1, gs, :].T
    rw = rw.astype(ml_dtypes.bfloat16)

    # Input-projection lhsT for layers 1,2: pw[k, l-1, g, d, m]
    pw = np.zeros((128, NL - 1, 4, 2, 64), np.float32)
    for l in (1, 2):
        for g in range(4):
            gs = slice(g * H, (g + 1) * H)
            for d in range(2):
                pw[:, l - 1, g, d, :] = w_ih[l, d, gs, :].T
    pw = pw.astype(ml_dtypes.bfloat16)

    # Layer-0 projection lhsT, both directions + bias rows, block-diagonal:
    # rows 0:8 fwd weights, row 8 fwd bias, rows 9:17 bwd weights, row 17 bwd
    l0w = np.zeros((18, 4, 128), np.float32)
    for g in range(4):
        gs = slice(g * H, (g + 1) * H)
        l0w[0:8, g, 0:64] = w_ih[0, 0, gs, 0:8].T
        l0w[8, g, 0:64] = bias[0, 0, gs]
        l0w[9:17, g, 64:128] = w_ih[0, 1, gs, 0:8].T
        l0w[17, g, 64:128] = bias[0, 1, gs]
    l0w = l0w.astype(ml_dtypes.bfloat16)

    # per-partition gate biases for layers 1,2 (fused dirs): br[p, l-1, g]
    br = np.zeros((128, NL - 1, 4), np.float32)
    for l in (1, 2):
        for g in range(4):
            gs = slice(g * H, (g + 1) * H)
            br[0:64, l - 1, g] = bias[l, 0, gs]
            br[64:128, l - 1, g] = bias[l, 1, gs]

    # residual rhs: wres[k, f] = w_res[f, k], row 8 = b_res
    wres = np.zeros((9, 128), np.float32)
    wres[0:8, :] = w_res.T
    wres[8, :] = b_res
    wres = wres.astype(ml_dtypes.bfloat16)

    # ones-column lhsT for the LN sum matmuls: onescube[p, j, m] = (m==j)/128
    onescube = np.zeros((128, SG, SG), np.float32)
    for j in range(SG):
        onescube[:, j, j] = 1.0 / D2
    onescube = onescube.astype(ml_dtypes.bfloat16)

    ident = np.eye(128, dtype=np.float32).astype(ml_dtypes.bfloat16)

    # Per-core transposed-augmented input (bf16): xaug[k, t, b]
    xaug_cores = []
    for c in range(ncores):
        xc = x[c * bc:(c + 1) * bc]              # (bc, T, 8)
        xa = np.empty((9, t_len, bc), np.float32)
        xa[0:8] = xc.transpose(2, 1, 0)
        xa[8] = 1.0
        xaug_cores.append(xa.astype(ml_dtypes.bfloat16))

    shared = dict(rw=rw, pw=pw, l0w=l0w, br=br, wres=wres,
                  onescube=onescube, ident=ident)
    return shared, xaug_cores


def _emit(nc, tc, ctx, D, apply_gb, bc, t_len):
    bk = bc // CHUNKS
    nb = bk // 128            # natural-layout 128-row blocks per chunk
    ngrp = t_len // SG

    sbC = ctx.enter_context(tc.tile_pool(name="consts", bufs=1))
    sbA = ctx.enter_context(tc.tile_pool(name="inps", bufs=1))
    sbB = ctx.enter_context(tc.tile_pool(name="work", bufs=1))
    sbS = ctx.enter_context(tc.tile_pool(name="state", bufs=1))
    ps = ctx.enter_context(tc.tile_pool(name="ps", bufs=1, space="PSUM"))

    def const_tile(shape, dtype, key):
        t = sbC.tile(shape, dtype, name=f"c_{key}", tag=f"c_{key}")
        nc.sync.dma_start(out=t, in_=D[key])
        return t

    rw_sb = const_tile([128, NL, 4, 128], BF16, "rw")
    pw_sb = const_tile([128, NL - 1, 4, 2, 64], BF16, "pw")
    l0w_sb = const_tile([18, 4, 128], BF16, "l0w")
    br_sb = const_tile([128, NL - 1, 4], F32, "br")
    wres_sb = const_tile([9, 128], BF16, "wres")
    ones_sb = const_tile([128, SG, SG], BF16, "onescube")
    ident_sb = const_tile([128, 128], BF16, "ident")
    gamma_sb = beta_sb = None
    if apply_gb:
        gamma_sb = const_tile([128, 128], F32, "gammab")
        beta_sb = const_tile([128, 128], F32, "betab")
    eps_sb = sbC.tile([128, 1], F32)
    nc.vector.memset(eps_sb, LN_EPS)

    O = [D[f"o{i}"] for i in range(NL)]
    xaug = D["xaug"]
    out_d = D["out"]

    cols = [slice(cc * bk, (cc + 1) * bk) for cc in range(CHUNKS)]

    # ---------------- LSTM layers ----------------

    def issue_group(l, cc, grp):
        k0 = grp * SG
        lo = t_len - k0 - SG
        hi = t_len - k0
        if l == 0:
            xa = sbA.tile([18, SG, bk], BF16, tag=f"inF{cc}", bufs=2, name="xa")
            nc.sync.dma_start(out=xa[0:9], in_=xaug[:, k0:k0 + SG, cols[cc]])
            nc.sync.dma_start(out=xa[9:18],
                              in_=xaug[:, lo:hi, cols[cc]][:, ::-1, :])
            return (xa, None)
        inF = sbA.tile([128, SG, bk], BF16, tag=f"inF{cc}", bufs=2, name="inF")
        nc.sync.dma_start(out=inF, in_=O[l - 1][:, k0:k0 + SG, cols[cc]])
        inB = sbA.tile([128, SG, bk], BF16, tag=f"inB{cc}", bufs=2, name="inB")
        nc.sync.dma_start(out=inB,
                          in_=O[l - 1][:, lo:hi, cols[cc]][:, ::-1, :])
        return (inF, inB)

    def emit_proj(l, cc, P, tiles, j, k):
        # input projections for step k (independent of the recurrence)
        stop = (k == 0)   # no recurrent matmul at k==0
        if l == 0:
            xa = tiles[0]
            for g in range(4):
                nc.tensor.matmul(P[:, g, :], l0w_sb[:, g, :], xa[:, j, :],
                                 start=True, stop=stop, skip_group_check=True)
        else:
            inF, inB = tiles
            for g in range(4):
                nc.tensor.matmul(P[0:64, g, :], pw_sb[:, l - 1, g, 0, :],
                                 inF[:, j, :], start=True, stop=stop,
                                 tile_position=(0, 0), skip_group_check=True)
                nc.tensor.matmul(P[64:128, g, :], pw_sb[:, l - 1, g, 1, :],
                                 inB[:, j, :], start=True, stop=stop,
                                 tile_position=(0, 64), skip_group_check=True)

    h_prev = [None] * CHUNKS
    c_st = [None] * CHUNKS
    stage_cur = [None] * CHUNKS

    for l in range(NL):
        pend = {}
        for cc in range(CHUNKS):
            pend[(cc, 0)] = issue_group(l, cc, 0)
        P_cur = [None] * CHUNKS
        for cc in range(CHUNKS):
            P_cur[cc] = ps.tile([128, 4, bk], F32, tag=f"p{cc}", name="P")
            emit_proj(l, cc, P_cur[cc], pend[(cc, 0)], 0, 0)

        for k in range(t_len):
            j = k % SG
            grp = k // SG
            if j == 0:
                if grp + 1 < ngrp:
                    for cc in range(CHUNKS):
                        pend[(cc, grp + 1)] = issue_group(l, cc, grp + 1)
                for cc in range(CHUNKS):
                    stage_cur[cc] = sbS.tile([128, SG, bk], BF16,
                                             tag=f"st{cc}", bufs=2,
                                             name="stage")
            S_os = [None] * CHUNKS
            for cc in range(CHUNKS):
                P = P_cur[cc]
                if k > 0:
                    for g in range(4):
                        nc.tensor.matmul(P[:, g, :], rw_sb[:, l, g, :],
                                         h_prev[cc], start=False, stop=True,
                                         skip_group_check=True)

                def bias(g):
                    if l == 0:
                        return 0.0
                    return br_sb[:, l - 1, g:g + 1]

                S_if = sbB.tile([128, 2, bk], F32, tag=f"sif{cc}", bufs=2,
                                name="S_if")
                S_g = sbB.tile([128, bk], F32, tag=f"sg{cc}", bufs=2,
                               name="S_g")
                S_o = sbB.tile([128, bk], BF16, tag=f"so{cc}", bufs=2,
                               name="S_o")
                nc.scalar.activation(out=S_if[:, 0, :], in_=P[:, GI, :],
                                     func=AF.Sigmoid, bias=bias(GI))
                nc.scalar.activation(out=S_g, in_=P[:, GG, :],
                                     func=AF.Tanh, bias=bias(GG))
                nc.scalar.activation(out=S_if[:, 1, :], in_=P[:, GF, :],
                                     func=AF.Sigmoid, bias=bias(GF))
                nc.scalar.activation(out=S_o, in_=P[:, GO, :],
                                     func=AF.Sigmoid, bias=bias(GO))
                if k == 0:
                    c = sbS.tile([128, bk], F32, tag=f"c{cc}", name="c")
                    c_st[cc] = c
                    nc.vector.tensor_mul(c, S_if[:, 0, :], S_g)
                else:
                    c = c_st[cc]
                    tmp = sbB.tile([128, bk], F32, tag=f"tmp{cc}", bufs=2,
                                   name="tmp")
                    nc.gpsimd.tensor_mul(tmp, S_if[:, 0, :], S_g)
                    nc.vector.tensor_mul(c, S_if[:, 1, :], c)
                    nc.vector.tensor_add(c, c, tmp)
                S_os[cc] = S_o
            for cc in range(CHUNKS):
                Tc = sbB.tile([128, bk], BF16, tag=f"tc{cc}", bufs=2,
                              name="Tc")
                nc.scalar.activation(out=Tc, in_=c_st[cc], func=AF.Tanh)
                hslot = stage_cur[cc][:, j, :]
                nc.vector.tensor_mul(hslot, S_os[cc], Tc)
                h_prev[cc] = hslot
            # next step's projections (prefetched past the h dependency)
            if k + 1 < t_len:
                jn = (k + 1) % SG
                gn = (k + 1) // SG
                for cc in range(CHUNKS):
                    P_cur[cc] = ps.tile([128, 4, bk], F32, tag=f"p{cc}",
                                        name="P")
                    emit_proj(l, cc, P_cur[cc], pend[(cc, gn)], jn, k + 1)
            if j == SG - 1:
                k0 = grp * SG
                lo = t_len - k0 - SG
                hi = t_len - k0
                for cc in range(CHUNKS):
                    nc.sync.dma_start(out=O[l][0:64, k0:k0 + SG, cols[cc]],
                                      in_=stage_cur[cc][0:64, :, :])
                    nc.sync.dma_start(
                        out=O[l][64:128, lo:hi, cols[cc]][:, ::-1, :],
                        in_=stage_cur[cc][64:128, :, :])
                if grp >= 1:
                    pend.pop((0, grp - 1), None)
                    pend.pop((1, grp - 1), None)

    # ---------------- final stage: relu + residual + LayerNorm ----------------
    # PSUM scratch per chunk reuses the gate tile (4 banks):
    #   slots 0/1: residual ping-pong, then bf16 z-transpose / stats regions
    #   slot 2: LN mean accumulator [0:8]   slot 3: LN sq-mean accumulator

    def issue_fin(cc, grp):
        t0 = grp * SG
        o2 = sbA.tile([128, SG, bk], BF16, tag=f"inF{cc}", bufs=2, name="o2")
        nc.sync.dma_start(out=o2, in_=O[NL - 1][:, t0:t0 + SG, cols[cc]])
        xa9 = sbA.tile([9, SG, bk], BF16, tag=f"inB{cc}", bufs=2, name="xa9")
        nc.sync.dma_start(out=xa9, in_=xaug[:, t0:t0 + SG, cols[cc]])
        relu = sbS.tile([128, SG, bk], BF16, tag=f"st{cc}", bufs=2,
                        name="relu")
        nc.gpsimd.tensor_scalar_max(relu, o2, 0.0)
        return relu, xa9

    fpend = {}
    for cc in range(CHUNKS):
        fpend[(cc, 0)] = issue_fin(cc, 0)

    for grp in range(ngrp):
        t0 = grp * SG
        if grp + 1 < ngrp:
            for cc in range(CHUNKS):
                fpend[(cc, grp + 1)] = issue_fin(cc, grp + 1)
        relu_t = [None] * CHUNKS
        scr_t = [None] * CHUNKS
        zs_t = [None] * CHUNKS
        xa9_t = [None] * CHUNKS
        zqs_t = [dict() for _ in range(CHUNKS)]
        for cc in range(CHUNKS):
            relu_t[cc], xa9_t[cc] = fpend.pop((cc, grp))
            scr_t[cc] = ps.tile([128, 4, bk], F32, tag=f"p{cc}", name="scr")
            zs_t[cc] = sbB.tile([128, SG, bk], BF16, tag=f"zs{cc}", name="zs")
        # the two chunks' z-chains are independent (separate PSUM tiles):
        # interleave them per step so both advance concurrently.  LN-sum
        # matmuls lag two steps so they never head-of-line block the next
        # residual on the PE queue.
        for jt in range(SG):
            for cc in range(CHUNKS):
                scr, zs = scr_t[cc], zs_t[cc]
                res = scr[:, jt % 2, :]
                nc.tensor.matmul(res, wres_sb, xa9_t[cc][:, jt, :],
                                 start=True, stop=True, skip_group_check=True)
                nc.vector.tensor_add(zs[:, jt, :], relu_t[cc][:, jt, :], res)
                zq = sbB.tile([128, bk], BF16, tag=f"zq{cc}", bufs=3,
                              name="zq")
                nc.gpsimd.tensor_mul(zq, zs[:, jt, :], zs[:, jt, :])
                zqs_t[cc][jt] = zq
                if jt >= 2:
                    jl = jt - 2
                    nc.tensor.matmul(scr[0:8, 2, :], ones_sb[:, jl, :],
                                     zs[:, jl, :], start=(jl == 0),
                                     stop=False, skip_group_check=True)
                    nc.tensor.matmul(scr[0:8, 3, :], ones_sb[:, jl, :],
                                     zqs_t[cc].pop(jl), start=(jl == 0),
                                     stop=False, skip_group_check=True)
        for cc in range(CHUNKS):
            scr, zs = scr_t[cc], zs_t[cc]
            for jl in (SG - 2, SG - 1):
                nc.tensor.matmul(scr[0:8, 2, :], ones_sb[:, jl, :],
                                 zs[:, jl, :], start=False,
                                 stop=(jl == SG - 1), skip_group_check=True)
                nc.tensor.matmul(scr[0:8, 3, :], ones_sb[:, jl, :],
                                 zqs_t[cc].pop(jl), start=False,
                                 stop=(jl == SG - 1), skip_group_check=True)
        # stats: mu/sqm [8, bk] -> natural layout [128, nb, 8]
        rstd_t = [None] * CHUNKS
        nmr_t = [None] * CHUNKS
        outst_t = [None] * CHUNKS
        for cc in range(CHUNKS):
            scr = scr_t[cc]
            musq = sbB.tile([8, 2, bk], BF16, tag=f"ms{cc}", name="musq")
            nc.scalar.activation(out=musq, in_=scr[0:8, 2:4, :],
                                 func=AF.Identity)
            sv = scr[:, 0, :].bitcast(BF16)      # [128, 2*bk] bf16 view
            for bi in range(nb):
                nc.tensor.matmul(sv[:, bi * 16:bi * 16 + 8],
                                 musq[:, 0, bi * 128:bi * 128 + 128],
                                 ident_sb[0:8, 0:8], is_transpose=True,
                                 start=True, stop=True, skip_group_check=True)
                nc.tensor.matmul(sv[:, bi * 16 + 8:bi * 16 + 16],
                                 musq[:, 1, bi * 128:bi * 128 + 128],
                                 ident_sb[0:8, 0:8], is_transpose=True,
                                 start=True, stop=True, skip_group_check=True)
            snat = sbB.tile([128, nb, 16], BF16, tag=f"sn{cc}", name="snat")
            nc.scalar.activation(out=snat,
                                 in_=sv[:, 0:nb * 16].rearrange(
                                     "p (a c) -> p a c", a=nb),
                                 func=AF.Identity)
            mu_nat = snat[:, :, 0:8]
            sq_nat = snat[:, :, 8:16]
            mu2 = sbB.tile([128, nb, 8], F32, tag=f"mu2{cc}", name="mu2")
            nc.vector.tensor_mul(mu2, mu_nat, mu_nat)
            var = sbB.tile([128, nb, 8], F32, tag=f"var{cc}", name="var")
            nc.vector.tensor_sub(var, sq_nat, mu2)
            sd = sbB.tile([128, nb, 8], F32, tag=f"sd{cc}", name="sd")
            nc.scalar.activation(out=sd, in_=var, func=AF.Sqrt,
                                 bias=eps_sb)
            rstd_t[cc] = sbB.tile([128, nb, 8], F32, tag=f"rstd{cc}",
                                  name="rstd")
            nc.vector.reciprocal(rstd_t[cc], sd)
            nmr_t[cc] = sbB.tile([128, nb, 8], F32, tag=f"nmr{cc}",
                                 name="nmr")
            nc.vector.scalar_tensor_tensor(nmr_t[cc], mu_nat, -1.0,
                                           rstd_t[cc],
                                           op0=OP.mult, op1=OP.mult)
            outst_t[cc] = [sbB.tile([128, SG, 128], F32, tag=f"os{cc}{bi}",
                                    name="outst") for bi in range(nb)]
        for jt in range(SG):
            for cc in range(CHUNKS):
                scr, zs = scr_t[cc], zs_t[cc]
                rstd, nmr = rstd_t[cc], nmr_t[cc]
                zv = scr[:, jt % 2, :].bitcast(BF16)   # [128, 2*bk] bf16
                for bi in range(nb):
                    b0 = bi * 128
                    nc.tensor.matmul(zv[:, b0:b0 + 128],
                                     zs[:, jt, b0:b0 + 128],
                                     ident_sb, is_transpose=True,
                                     start=True, stop=True,
                                     skip_group_check=True)
                for bi in range(nb):
                    b0 = bi * 128
                    dst = outst_t[cc][bi][:, jt, :]
                    if bi % 2 == 0:
                        nc.scalar.activation(out=dst, in_=zv[:, b0:b0 + 128],
                                             func=AF.Identity,
                                             scale=rstd[:, bi, jt:jt + 1],
                                             bias=nmr[:, bi, jt:jt + 1])
                    else:
                        nc.vector.tensor_scalar(dst, zv[:, b0:b0 + 128],
                                                rstd[:, bi, jt:jt + 1],
                                                nmr[:, bi, jt:jt + 1],
                                                op0=OP.mult, op1=OP.add)
                    if apply_gb:
                        nc.vector.tensor_mul(dst, dst, gamma_sb)
                        nc.vector.tensor_add(dst, dst, beta_sb)
        for cc in range(CHUNKS):
            for bi in range(nb):
                b0 = cc * bk + bi * 128
                nc.sync.dma_start(out=out_d[b0:b0 + 128, t0:t0 + SG, :],
                                  in_=outst_t[cc][bi])


def build(apply_gb=False, bc=BC, t_len=T, num_devices=NCORES):
    nc = bacc.Bacc("TRN2", target_bir_lowering=False, debug=False,
                   num_devices=num_devices)
    D = {}

    def inp(name, shape, dtype=F32):
        D[name] = nc.dram_tensor(name, shape, dtype, kind="ExternalInput").ap()

    inp("xaug", [9, t_len, bc], BF16)
    inp("rw", [128, NL, 4, 128], BF16)
    inp("pw", [128, NL - 1, 4, 2, 64], BF16)
    inp("l0w", [18, 4, 128], BF16)
    inp("br", [128, NL - 1, 4])
    inp("wres", [9, 128], BF16)
    inp("onescube", [128, SG, SG], BF16)
    inp("ident", [128, 128], BF16)
    if apply_gb:
        inp("gammab", [128, 128])
        inp("betab", [128, 128])
    for i in range(NL):
        D[f"o{i}"] = nc.dram_tensor(f"o{i}", [128, t_len, bc], BF16).ap()
    D["out"] = nc.dram_tensor("out", [bc, t_len, 128], F32,
                              kind="ExternalOutput").ap()

    with tile.TileContext(nc) as tc:
        with ExitStack() as ctx:
            _emit(nc, tc, ctx, D, apply_gb, bc, t_len)
    nc.compile()
    return nc


_BUILD_CACHE = {}


def kernel(x, w_ih, w_hh, b_ih, b_hh, w_res, b_res, ln_gamma, ln_beta):
    ln_gamma = np.asarray(ln_gamma, np.float32)
    ln_beta = np.asarray(ln_beta, np.float32)
    apply_gb = not (np.all(ln_gamma == 1.0) and np.all(ln_beta == 0.0))

    shared, xaug_cores = _host_prep(x, w_ih, w_hh, b_ih, b_hh, w_res, b_res,
                                    NCORES, BC)
    if apply_gb not in _BUILD_CACHE:
        _BUILD_CACHE[apply_gb] = build(apply_gb)
    nc = _BUILD_CACHE[apply_gb]

    in_maps = []
    for c in range(NCORES):
        m = dict(shared)
        m["xaug"] = xaug_cores[c]
        if apply_gb:
            m["gammab"] = np.ascontiguousarray(
                np.broadcast_to(ln_gamma, (128, 128)).astype(np.float32))
            m["betab"] = np.ascontiguousarray(
                np.broadcast_to(ln_beta, (128, 128)).astype(np.float32))
        in_maps.append(m)

    res = run_bass_kernel_spmd(nc, in_maps, core_ids=list(range(NCORES)))
    out = np.concatenate([res.results[c]["out"] for c in range(NCORES)], axis=0)
    return np.ascontiguousarray(out.astype(np.float32))


# revision 11
# speedup vs baseline: 1.0166x; 1.0166x over previous
"""Trainium2 Bass kernel for a 3-layer BiLSTM + ReLU + residual + LayerNorm.

Pure data parallel over 8 cores (1024 batch rows per core).  Per core:

  * Transposed on-chip layout: features on partitions (fwd 0:64 / bwd 64:128
    fused), batch on the free dim, processed as two interleaved 512-row
    chunks so the engines ping-pong across the sequential dependency chain.
  * All matmuls are bf16 (1 cycle/row on the PE): the per-gate recurrent
    matmul is one block-diagonal 128x128; the layer-0 input projection is a
    single K=18 matmul per gate (both directions + bias rows folded in);
    layers 1-2 use two M=64 column-half matmuls per gate.  Recurrent
    matmuls are emitted at the head of each step and the next step's
    projections are emitted after the activations, so the PE queue is never
    head-of-line blocked on the h(t-1) dependency.
  * ScalarE runs the 4 gate activations (per-gate bias APs, sigmoid/tanh)
    plus tanh(c); i*g runs on GpSimd; f*c, the accumulate, and h=o*tanh(c)
    run on VectorE.  h is produced directly as bf16 into an 8-step staging
    ring that doubles as the recurrent-matmul rhs and the layer-output DMA
    source.
  * All HBM traffic is strip-batched (8 timesteps per DMA, backward halves
    via negative-stride APs): ~300 DMAs total instead of ~2300, which keeps
    the SP sequencer and the shared HWDGE path off the critical path.
  * Final stage per 8-step strip: residual via one K=9 matmul, z = relu+res
    on VectorE (bf16), z^2 on GpSimd, LN mean/sq-mean via ones-column
    accumulating matmuls (pre-scaled 1/128), tiny PE transposes to move the
    stats to natural layout, then a per-batch-row normalize split between
    ScalarE (scale/bias APs) and VectorE reading the PE-transposed z
    directly from PSUM.  relu runs one strip ahead on GpSimd; the two
    chunks' z-chains are interleaved per step (independent PSUM tiles) and
    the LN-sum matmuls lag two steps so they never head-of-line block the
    next residual matmul on the PE queue.

Cost-model time: ~1.64 ms vs the 3.27 ms fp32 baseline; max rel err ~7e-3
(bf16 recurrence) against the fp32 reference, within the 2e-2 gate.
"""

from contextlib import ExitStack

import numpy as np
import ml_dtypes

import concourse.bacc as bacc
import concourse.tile as tile
from concourse import mybir
from concourse.bass_utils import run_bass_kernel_spmd

F32 = mybir.dt.float32
BF16 = mybir.dt.bfloat16
AF = mybir.ActivationFunctionType
OP = mybir.AluOpType

NCORES = 8
BC = 1024               # batch rows per core
CHUNKS = 2
T = 64
H = 64
NL = 3
D2 = 2 * H              # 128
LN_EPS = 1e-5
SG = 8                  # timesteps per DMA strip group

# PyTorch gate order: i, f, g, o
GI, GF, GG, GO = 0, 1, 2, 3


def _host_prep(x, w_ih, w_hh, b_ih, b_hh, w_res, b_res, ncores, bc):
    """Matmul-ready weight layouts (shared across cores) + per-core inputs."""
    x = np.asarray(x, np.float32)
    w_ih = np.asarray(w_ih, np.float32)
    w_hh = np.asarray(w_hh, np.float32)
    bias = np.asarray(b_ih, np.float32) + np.asarray(b_hh, np.float32)  # (NL,2,4H)
    w_res = np.asarray(w_res, np.float32)
    b_res = np.asarray(b_res, np.float32)
    t_len = x.shape[1]

    # Recurrent lhsT, K-major: rw[k, l, g, m] (block-diagonal over directions)
    rw = np.zeros((128, NL, 4, 128), np.float32)
    for l in range(NL):
        for g in range(4):
            gs = slice(g * H, (g + 1) * H)
            rw[0:64, l, g, 0:64] = w_hh[l, 0, gs, :].T
            rw[64:128, l, g, 64:128] = w_hh[l, 1, gs, :].T
    rw = rw.astype(ml_dtypes.bfloat16)

    # Input-projection lhsT for layers 1,2: pw[k, l-1, g, d, m]
    pw = np.zeros((128, NL - 1, 4, 2, 64), np.float32)
    for l in (1, 2):
        for g in range(4):
            gs = slice(g * H, (g + 1) * H)
            for d in range(2):
                pw[:, l - 1, g, d, :] = w_ih[l, d, gs, :].T
    pw = pw.astype(ml_dtypes.bfloat16)

    # Layer-0 projection lhsT, both directions + bias rows, block-diagonal:
    # rows 0:8 fwd weights, row 8 fwd bias, rows 9:17 bwd weights, row 17 bwd
    l0w = np.zeros((18, 4, 128), np.float32)
    for g in range(4):
        gs = slice(g * H, (g + 1) * H)
        l0w[0:8, g, 0:64] = w_ih[0, 0, gs, 0:8].T
        l0w[8, g, 0:64] = bias[0, 0, gs]
        l0w[9:17, g, 64:128] = w_ih[0, 1, gs, 0:8].T
        l0w[17, g, 64:128] = bias[0, 1, gs]
    l0w = l0w.astype(ml_dtypes.bfloat16)

    # per-partition gate biases for layers 1,2 (fused dirs): br[p, l-1, g]
    br = np.zeros((128, NL - 1, 4), np.float32)
    for l in (1, 2):
        for g in range(4):
            gs = slice(g * H, (g + 1) * H)
            br[0:64, l - 1, g] = bias[l, 0, gs]
            br[64:128, l - 1, g] = bias[l, 1, gs]

    # residual rhs: wres[k, f] = w_res[f, k], row 8 = b_res
    wres = np.zeros((9, 128), np.float32)
    wres[0:8, :] = w_res.T
    wres[8, :] = b_res
    wres = wres.astype(ml_dtypes.bfloat16)

    # ones-column lhsT for the LN sum matmuls: onescube[p, j, m] = (m==j)/128
    onescube = np.zeros((128, SG, SG), np.float32)
    for j in range(SG):
        onescube[:, j, j] = 1.0 / D2
    onescube = onescube.astype(ml_dtypes.bfloat16)

    ident = np.eye(128, dtype=np.float32).astype(ml_dtypes.bfloat16)

    # Per-core transposed-augmented input (bf16): xaug[k, t, b]
    xaug_cores = []
    for c in range(ncores):
        xc = x[c * bc:(c + 1) * bc]              # (bc, T, 8)
        xa = np.empty((9, t_len, bc), np.float32)
        xa[0:8] = xc.transpose(2, 1, 0)
        xa[8] = 1.0
        xaug_cores.append(xa.astype(ml_dtypes.bfloat16))

    shared = dict(rw=rw, pw=pw, l0w=l0w, br=br, wres=wres,
                  onescube=onescube, ident=ident)
    return shared, xaug_cores


def _emit(nc, tc, ctx, D, apply_gb, bc, t_len):
    bk = bc // CHUNKS
    nb = bk // 128            # natural-layout 128-row blocks per chunk
    ngrp = t_len // SG

    sbC = ctx.enter_context(tc.tile_pool(name="consts", bufs=1))
    sbA = ctx.enter_context(tc.tile_pool(name="inps", bufs=1))
    sbB = ctx.enter_context(tc.tile_pool(name="work", bufs=1))
    sbS = ctx.enter_context(tc.tile_pool(name="state", bufs=1))
    ps = ctx.enter_context(tc.tile_pool(name="ps", bufs=1, space="PSUM"))

    def const_tile(shape, dtype, key):
        t = sbC.tile(shape, dtype, name=f"c_{key}", tag=f"c_{key}")
        nc.sync.dma_start(out=t, in_=D[key])
        return t

    rw_sb = const_tile([128, NL, 4, 128], BF16, "rw")
    pw_sb = const_tile([128, NL - 1, 4, 2, 64], BF16, "pw")
    l0w_sb = const_tile([18, 4, 128], BF16, "l0w")
    br_sb = const_tile([128, NL - 1, 4], F32, "br")
    wres_sb = const_tile([9, 128], BF16, "wres")
    ones_sb = const_tile([128, SG, SG], BF16, "onescube")
    ident_sb = const_tile([128, 128], BF16, "ident")
    gamma_sb = beta_sb = None
    if apply_gb:
        gamma_sb = const_tile([128, 128], F32, "gammab")
        beta_sb = const_tile([128, 128], F32, "betab")
    eps_sb = sbC.tile([128, 1], F32)
    nc.vector.memset(eps_sb, LN_EPS)

    O = [D[f"o{i}"] for i in range(NL)]
    xaug = D["xaug"]
    out_d = D["out"]

    cols = [slice(cc * bk, (cc + 1) * bk) for cc in range(CHUNKS)]

    # ---------------- LSTM layers ----------------

    def issue_group(l, cc, grp):
        k0 = grp * SG
        lo = t_len - k0 - SG
        hi = t_len - k0
        if l == 0:
            xa = sbA.tile([18, SG, bk], BF16, tag=f"inF{cc}", bufs=2, name="xa")
            nc.sync.dma_start(out=xa[0:9], in_=xaug[:, k0:k0 + SG, cols[cc]])
            nc.sync.dma_start(out=xa[9:18],
                              in_=xaug[:, lo:hi, cols[cc]][:, ::-1, :])
            return (xa, None)
        inF = sbA.tile([128, SG, bk], BF16, tag=f"inF{cc}", bufs=2, name="inF")
        nc.sync.dma_start(out=inF, in_=O[l - 1][:, k0:k0 + SG, cols[cc]])
        inB = sbA.tile([128, SG, bk], BF16, tag=f"inB{cc}", bufs=2, name="inB")
        nc.sync.dma_start(out=inB,
                          in_=O[l - 1][:, lo:hi, cols[cc]][:, ::-1, :])
        return (inF, inB)

    def emit_proj(l, cc, P, tiles, j, k):
        # input projections for step k (independent of the recurrence)
        stop = (k == 0)   # no recurrent matmul at k==0
        if l == 0:
            xa = tiles[0]
            for g in range(4):
                nc.tensor.matmul(P[:, g, :], l0w_sb[:, g, :], xa[:, j, :],
                                 start=True, stop=stop, skip_group_check=True)
        else:
            inF, inB = tiles
            for g in range(4):
                nc.tensor.matmul(P[0:64, g, :], pw_sb[:, l - 1, g, 0, :],
                                 inF[:, j, :], start=True, stop=stop,
                                 tile_position=(0, 0), skip_group_check=True)
                nc.tensor.matmul(P[64:128, g, :], pw_sb[:, l - 1, g, 1, :],
                                 inB[:, j, :], start=True, stop=stop,
                                 tile_position=(0, 64), skip_group_check=True)

    h_prev = [None] * CHUNKS
    c_st = [None] * CHUNKS
    stage_cur = [None] * CHUNKS

    for l in range(NL):
        pend = {}
        for cc in range(CHUNKS):
            pend[(cc, 0)] = issue_group(l, cc, 0)
        P_cur = [None] * CHUNKS
        for cc in range(CHUNKS):
            P_cur[cc] = ps.tile([128, 4, bk], F32, tag=f"p{cc}", name="P")
            emit_proj(l, cc, P_cur[cc], pend[(cc, 0)], 0, 0)

        for k in range(t_len):
            j = k % SG
            grp = k // SG
            if j == 0:
                if grp + 1 < ngrp:
                    for cc in range(CHUNKS):
                        pend[(cc, grp + 1)] = issue_group(l, cc, grp + 1)
                for cc in range(CHUNKS):
                    stage_cur[cc] = sbS.tile([128, SG, bk], BF16,
                                             tag=f"st{cc}", bufs=2,
                                             name="stage")
            S_os = [None] * CHUNKS
            for cc in range(CHUNKS):
                P = P_cur[cc]
                if k > 0:
                    for g in range(4):
                        nc.tensor.matmul(P[:, g, :], rw_sb[:, l, g, :],
                                         h_prev[cc], start=False, stop=True,
                                         skip_group_check=True)

                def bias(g):
                    if l == 0:
                        return 0.0
                    return br_sb[:, l - 1, g:g + 1]

                S_if = sbB.tile([128, 2, bk], F32, tag=f"sif{cc}", bufs=2,
                                name="S_if")
                S_g = sbB.tile([128, bk], F32, tag=f"sg{cc}", bufs=2,
                               name="S_g")
                S_o = sbB.tile([128, bk], BF16, tag=f"so{cc}", bufs=2,
                               name="S_o")
                if l == 0:
                    # layer-0 biases ride the projection; i and f (adjacent
                    # PSUM slots) share one wider sigmoid
                    nc.scalar.activation(out=S_if, in_=P[:, GI:GF + 1, :],
                                         func=AF.Sigmoid)
                    nc.scalar.activation(out=S_g, in_=P[:, GG, :],
                                         func=AF.Tanh)
                else:
                    nc.scalar.activation(out=S_if[:, 0, :], in_=P[:, GI, :],
                                         func=AF.Sigmoid, bias=bias(GI))
                    nc.scalar.activation(out=S_g, in_=P[:, GG, :],
                                         func=AF.Tanh, bias=bias(GG))
                    nc.scalar.activation(out=S_if[:, 1, :], in_=P[:, GF, :],
                                         func=AF.Sigmoid, bias=bias(GF))
                nc.scalar.activation(out=S_o, in_=P[:, GO, :],
                                     func=AF.Sigmoid, bias=bias(GO))
                if k == 0:
                    c = sbS.tile([128, bk], F32, tag=f"c{cc}", name="c")
                    c_st[cc] = c
                    nc.vector.tensor_mul(c, S_if[:, 0, :], S_g)
                else:
                    c = c_st[cc]
                    tmp = sbB.tile([128, bk], F32, tag=f"tmp{cc}", bufs=2,
                                   name="tmp")
                    nc.gpsimd.tensor_mul(tmp, S_if[:, 0, :], S_g)
                    nc.vector.tensor_mul(c, S_if[:, 1, :], c)
                    nc.vector.tensor_add(c, c, tmp)
                S_os[cc] = S_o
            for cc in range(CHUNKS):
                Tc = sbB.tile([128, bk], BF16, tag=f"tc{cc}", bufs=2,
                              name="Tc")
                nc.scalar.activation(out=Tc, in_=c_st[cc], func=AF.Tanh)
                hslot = stage_cur[cc][:, j, :]
                nc.vector.tensor_mul(hslot, S_os[cc], Tc)
                h_prev[cc] = hslot
            # next step's projections (prefetched past the h dependency)
            if k + 1 < t_len:
                jn = (k + 1) % SG
                gn = (k + 1) // SG
                for cc in range(CHUNKS):
                    P_cur[cc] = ps.tile([128, 4, bk], F32, tag=f"p{cc}",
                                        name="P")
                    emit_proj(l, cc, P_cur[cc], pend[(cc, gn)], jn, k + 1)
            if j == SG - 1:
                k0 = grp * SG
                lo = t_len - k0 - SG
                hi = t_len - k0
                for cc in range(CHUNKS):
                    nc.sync.dma_start(out=O[l][0:64, k0:k0 + SG, cols[cc]],
                                      in_=stage_cur[cc][0:64, :, :])
                    nc.sync.dma_start(
                        out=O[l][64:128, lo:hi, cols[cc]][:, ::-1, :],
                        in_=stage_cur[cc][64:128, :, :])
                if grp >= 1:
                    pend.pop((0, grp - 1), None)
                    pend.pop((1, grp - 1), None)

    # ---------------- final stage: relu + residual + LayerNorm ----------------
    # PSUM scratch per chunk reuses the gate tile (4 banks):
    #   slots 0/1: residual ping-pong, then bf16 z-transpose / stats regions
    #   slot 2: LN mean accumulator [0:8]   slot 3: LN sq-mean accumulator

    def issue_fin(cc, grp):
        t0 = grp * SG
        o2 = sbA.tile([128, SG, bk], BF16, tag=f"inF{cc}", bufs=2, name="o2")
        nc.sync.dma_start(out=o2, in_=O[NL - 1][:, t0:t0 + SG, cols[cc]])
        xa9 = sbA.tile([9, SG, bk], BF16, tag=f"inB{cc}", bufs=2, name="xa9")
        nc.sync.dma_start(out=xa9, in_=xaug[:, t0:t0 + SG, cols[cc]])
        relu = sbS.tile([128, SG, bk], BF16, tag=f"st{cc}", bufs=2,
                        name="relu")
        nc.gpsimd.tensor_scalar_max(relu, o2, 0.0)
        return relu, xa9

    fpend = {}
    for cc in range(CHUNKS):
        fpend[(cc, 0)] = issue_fin(cc, 0)

    for grp in range(ngrp):
        t0 = grp * SG
        if grp + 1 < ngrp:
            for cc in range(CHUNKS):
                fpend[(cc, grp + 1)] = issue_fin(cc, grp + 1)
        relu_t = [None] * CHUNKS
        scr_t = [None] * CHUNKS
        zs_t = [None] * CHUNKS
        xa9_t = [None] * CHUNKS
        zqs_t = [dict() for _ in range(CHUNKS)]
        for cc in range(CHUNKS):
            relu_t[cc], xa9_t[cc] = fpend.pop((cc, grp))
            scr_t[cc] = ps.tile([128, 4, bk], F32, tag=f"p{cc}", name="scr")
            zs_t[cc] = sbB.tile([128, SG, bk], BF16, tag=f"zs{cc}", name="zs")
        # the two chunks' z-chains are independent (separate PSUM tiles):
        # interleave them per step so both advance concurrently.  LN-sum
        # matmuls lag two steps so they never head-of-line block the next
        # residual on the PE queue.
        for jt in range(SG):
            for cc in range(CHUNKS):
                scr, zs = scr_t[cc], zs_t[cc]
                res = scr[:, jt % 2, :]
                nc.tensor.matmul(res, wres_sb, xa9_t[cc][:, jt, :],
                                 start=True, stop=True, skip_group_check=True)
                nc.vector.tensor_add(zs[:, jt, :], relu_t[cc][:, jt, :], res)
                zq = sbB.tile([128, bk], BF16, tag=f"zq{cc}", bufs=3,
                              name="zq")
                nc.gpsimd.tensor_mul(zq, zs[:, jt, :], zs[:, jt, :])
                zqs_t[cc][jt] = zq
                if jt >= 2:
                    jl = jt - 2
                    nc.tensor.matmul(scr[0:8, 2, :], ones_sb[:, jl, :],
                                     zs[:, jl, :], start=(jl == 0),
                                     stop=False, skip_group_check=True)
                    nc.tensor.matmul(scr[0:8, 3, :], ones_sb[:, jl, :],
                                     zqs_t[cc].pop(jl), start=(jl == 0),
                                     stop=False, skip_group_check=True)
        for cc in range(CHUNKS):
            scr, zs = scr_t[cc], zs_t[cc]
            for jl in (SG - 2, SG - 1):
                nc.tensor.matmul(scr[0:8, 2, :], ones_sb[:, jl, :],
                                 zs[:, jl, :], start=False,
                                 stop=(jl == SG - 1), skip_group_check=True)
                nc.tensor.matmul(scr[0:8, 3, :], ones_sb[:, jl, :],
                                 zqs_t[cc].pop(jl), start=False,
                                 stop=(jl == SG - 1), skip_group_check=True)
        # stats: mu/sqm [8, bk] -> natural layout [128, nb, 8]
        rstd_t = [None] * CHUNKS
        nmr_t = [None] * CHUNKS
        outst_t = [None] * CHUNKS
        for cc in range(CHUNKS):
            scr = scr_t[cc]
            musq = sbB.tile([8, 2, bk], BF16, tag=f"ms{cc}", name="musq")
            nc.scalar.activation(out=musq, in_=scr[0:8, 2:4, :],
                                 func=AF.Identity)
            sv = scr[:, 0, :].bitcast(BF16)      # [128, 2*bk] bf16 view
            for bi in range(nb):
                nc.tensor.matmul(sv[:, bi * 16:bi * 16 + 8],
                                 musq[:, 0, bi * 128:bi * 128 + 128],
                                 ident_sb[0:8, 0:8], is_transpose=True,
                                 start=True, stop=True, skip_group_check=True)
                nc.tensor.matmul(sv[:, bi * 16 + 8:bi * 16 + 16],
                                 musq[:, 1, bi * 128:bi * 128 + 128],
                                 ident_sb[0:8, 0:8], is_transpose=True,
                                 start=True, stop=True, skip_group_check=True)
            snat = sbB.tile([128, nb, 16], BF16, tag=f"sn{cc}", name="snat")
            nc.scalar.activation(out=snat,
                                 in_=sv[:, 0:nb * 16].rearrange(
                                     "p (a c) -> p a c", a=nb),
                                 func=AF.Identity)
            mu_nat = snat[:, :, 0:8]
            sq_nat = snat[:, :, 8:16]
            mu2 = sbB.tile([128, nb, 8], F32, tag=f"mu2{cc}", name="mu2")
            nc.vector.tensor_mul(mu2, mu_nat, mu_nat)
            var = sbB.tile([128, nb, 8], F32, tag=f"var{cc}", name="var")
            nc.vector.tensor_sub(var, sq_nat, mu2)
            sd = sbB.tile([128, nb, 8], F32, tag=f"sd{cc}", name="sd")
            nc.scalar.activation(out=sd, in_=var, func=AF.Sqrt,
                                 bias=eps_sb)
            rstd_t[cc] = sbB.tile([128, nb, 8], F32, tag=f"rstd{cc}",
                                  name="rstd")
            nc.vector.reciprocal(rstd_t[cc], sd)
            nmr_t[cc] = sbB.tile([128, nb, 8], F32, tag=f"nmr{cc}",
                                 name="nmr")
            nc.vector.scalar_tensor_tensor(nmr_t[cc], mu_nat, -1.0,
                                           rstd_t[cc],
                                           op0=OP.mult, op1=OP.mult)
            outst_t[cc] = [sbB.tile([128, SG, 128], F32, tag=f"os{cc}{bi}",
                                    name="outst") for bi in range(nb)]
        for jt in range(SG):
            for cc in range(CHUNKS):
                scr, zs = scr_t[cc], zs_t[cc]
                rstd, nmr = rstd_t[cc], nmr_t[cc]
                zv = scr[:, jt % 2, :].bitcast(BF16)   # [128, 2*bk] bf16
                for bi in range(nb):
                    b0 = bi * 128
                    nc.tensor.matmul(zv[:, b0:b0 + 128],
                                     zs[:, jt, b0:b0 + 128],
                                     ident_sb, is_transpose=True,
                                     start=True, stop=True,
                                     skip_group_check=True)
                for bi in range(nb):
                    b0 = bi * 128
                    dst = outst_t[cc][bi][:, jt, :]
                    if bi % 2 == 0:
                        nc.scalar.activation(out=dst, in_=zv[:, b0:b0 + 128],
                                             func=AF.Identity,
                                             scale=rstd[:, bi, jt:jt + 1],
                                             bias=nmr[:, bi, jt:jt + 1])
                    else:
                        nc.vector.tensor_scalar(dst, zv[:, b0:b0 + 128],
                                                rstd[:, bi, jt:jt + 1],
                                                nmr[:, bi, jt:jt + 1],
                                                op0=OP.mult, op1=OP.add)
                    if apply_gb:
                        nc.vector.tensor_mul(dst, dst, gamma_sb)
                        nc.vector.tensor_add(dst, dst, beta_sb)
        for cc in range(CHUNKS):
            for bi in range(nb):
                b0 = cc * bk + bi * 128
                nc.sync.dma_start(out=out_d[b0:b0 + 128, t0:t0 + SG, :],
                                  in_=outst_t[cc][bi])


def build(apply_gb=False, bc=BC, t_len=T, num_devices=NCORES):
    nc = bacc.Bacc("TRN2", target_bir_lowering=False, debug=False,
                   num_devices=num_devices)
    D = {}

    def inp(name, shape, dtype=F32):
        D[name] = nc.dram_tensor(name, shape, dtype, kind="ExternalInput").ap()

    inp("xaug", [9, t_len, bc], BF16)
    inp("rw", [128, NL, 4, 128], BF16)
    inp("pw", [128, NL - 1, 4, 2, 64], BF16)
    inp("l0w", [18, 4, 128], BF16)
    inp("br", [128, NL - 1, 4])
    inp("wres", [9, 128], BF16)
    inp("onescube", [128, SG, SG], BF16)
    inp("ident", [128, 128], BF16)
    if apply_gb:
        inp("gammab", [128, 128])
        inp("betab", [128, 128])
    for i in range(NL):
        D[f"o{i}"] = nc.dram_tensor(f"o{i}", [128, t_len, bc], BF16).ap()
    D["out"] = nc.dram_tensor("out", [bc, t_len, 128], F32,
                              kind="ExternalOutput").ap()

    with tile.TileContext(nc) as tc:
        with ExitStack() as ctx:
            _emit(nc, tc, ctx, D, apply_gb, bc, t_len)
    nc.compile()
    return nc


_BUILD_CACHE = {}


def kernel(x, w_ih, w_hh, b_ih, b_hh, w_res, b_res, ln_gamma, ln_beta):
    ln_gamma = np.asarray(ln_gamma, np.float32)
    ln_beta = np.asarray(ln_beta, np.float32)
    apply_gb = not (np.all(ln_gamma == 1.0) and np.all(ln_beta == 0.0))

    shared, xaug_cores = _host_prep(x, w_ih, w_hh, b_ih, b_hh, w_res, b_res,
                                    NCORES, BC)
    if apply_gb not in _BUILD_CACHE:
        _BUILD_CACHE[apply_gb] = build(apply_gb)
    nc = _BUILD_CACHE[apply_gb]

    in_maps = []
    for c in range(NCORES):
        m = dict(shared)
        m["xaug"] = xaug_cores[c]
        if apply_gb:
            m["gammab"] = np.ascontiguousarray(
                np.broadcast_to(ln_gamma, (128, 128)).astype(np.float32))
            m["betab"] = np.ascontiguousarray(
                np.broadcast_to(ln_beta, (128, 128)).astype(np.float32))
        in_maps.append(m)

    res = run_bass_kernel_spmd(nc, in_maps, core_ids=list(range(NCORES)))
    out = np.concatenate([res.results[c]["out"] for c in range(NCORES)], axis=0)
    return np.ascontiguousarray(out.astype(np.float32))
